# revision 17
# baseline (speedup 1.0000x reference)
"""T5-style causal multi-head attention (B=4, S=2048, E=1024, H=16, D=64)
on 8 NeuronCores. Sharding: core c handles batch c//2 and head half c%2
(8 heads). Host sums the two row-parallel partial output projections per
batch.  HW exec ~342us (baseline 614us).

Key optimizations over the original:
- The T5 bias saturates at bucket 31 for distance >= 113; that far-field
  value is constant per head across all keys of a query row, so it
  CANCELS IN SOFTMAX. Far blocks need no bias at all; only near blocks
  (mi <= 4) add a shifted table 8*(bias[bucket]-bias[31]) (+ mask -240)
  via a bf16 identity-matmul PSUM preload. The near table for all 4 head
  pairs stays SBUF-resident (no per-hp DMA bubbles).
- x is transposed on the HOST; stage 1 DMAs x^T tiles directly and runs
  only projection matmuls (no PE transposes, no PSUM->SBUF copy storm).
  All projection inputs/weights in bf16 (V is stored bf16 downstream
  anyway); projection drains run on the otherwise-idle ACT engine.
- Both heads' scores accumulate in one 2-bank PSUM tile [128,1024]; ONE
  ACT instruction does exp for both heads. Diagonal-straddling blocks
  are trimmed to the valid query range (w0) in QK, exp and PV.
- PV is issued one block behind QK so the PE never stalls on the exp.
- Softmax denominators come free via a ones-column in the PV lhsT
  (m=65); per (hp,qc) the den row is reciprocated on DVE, broadcast via
  a DRAM round-trip, and fused into the O^T drain multiply.
"""
import sys

sys.path.insert(0, "/opt/trn_rl_repo")

import numpy as np
import ml_dtypes

import concourse.bass as bass
import concourse.mybir as mybir
import concourse.tile as tile
from concourse import bacc
from concourse.bass_utils import run_bass_kernel_spmd
from concourse.masks import make_identity

F32, F32R, BF16 = mybir.dt.float32, mybir.dt.float32r, mybir.dt.bfloat16
F8 = mybir.dt.float8e4
AF = mybir.ActivationFunctionType

B, S, E, H, D = 4, 2048, 1024, 16, 64
HL = H // 2          # heads per core
HD = HL * D          # 512, per-core head dims
NUM_BUCKETS, MAX_DISTANCE = 32, 128
NEG8 = np.float32(-240.0)   # min-ish of fp8 e4m3 (IEEE): kills exp after /8
NT = S // 128        # 16 token blocks
NE = E // 8 // 16    # placeholder; real NE below
NE = E // 128        # 8 embed chunks
NQ = 4               # token quads (512 tokens each)

_NC_CACHE = {}


# ---------------------------------------------------------------- host side

def _np_bucket(distance):
    """Mirror reference._relative_position_bucket for causal (distance>=0),
    float32 arithmetic like jnp."""
    max_exact = NUM_BUCKETS // 2  # 16
    is_small = distance < max_exact
    safe = np.maximum(distance, 1).astype(np.float32)
    log_scale = np.log(safe / np.float32(max_exact)).astype(np.float32) / np.float32(
        np.log(np.float32(MAX_DISTANCE / max_exact))
    )
    large = max_exact + (log_scale * np.float32(NUM_BUCKETS - max_exact)).astype(
        np.int32
    )
    large = np.minimum(large, NUM_BUCKETS - 1)
    return np.where(is_small, distance, large)


def _build_btab_near(rel_bias_half):
    """rel_bias_half [8, 32] -> near-table [128 k, 4 hp, 5 mi, 2 h, 512 q]
    fp8, holding 8*(bias[bucket] - bias[31]) for valid, -240 for masked.
    The -bias[31] shift is the constant far-field bias, which cancels in
    softmax. m-index mi = (4*qc - kb) + 3; only mi <= 4 blocks need it."""
    rb = np.asarray(rel_bias_half, dtype=np.float32)        # [8, 32]
    qq = np.arange(512)[None, :]
    kk = np.arange(128)[:, None]
    tiles = []
    for mi in range(5):
        m = mi - 3
        dd = 128 * m + qq - kk                              # [128, 512]
        bucket = _np_bucket(np.maximum(dd, 0))
        vals = 8.0 * (rb[:, bucket] - rb[:, 31][:, None, None])   # [8,128,512]
        vals = np.where(dd[None] >= 0, vals, NEG8)
        tiles.append(vals.astype(np.float32))
    t = np.stack(tiles, axis=0)                             # [5, 8h, 128, 512]
    t = t.reshape(5, 4, 2, 128, 512).transpose(3, 1, 0, 2, 4)  # [128,4,5,2,512]
    return np.ascontiguousarray(t).astype(ml_dtypes.float8_e4m3)


def make_in_maps(inputs_q, inputs_kv, Wq, Wk, Wv, Wo, rel_bias):
    inputs_q = np.asarray(inputs_q, dtype=np.float32)
    inputs_kv = np.asarray(inputs_kv, dtype=np.float32)
    Wq = np.asarray(Wq, dtype=np.float32)
    Wk = np.asarray(Wk, dtype=np.float32)
    Wv = np.asarray(Wv, dtype=np.float32)
    Wo = np.asarray(Wo, dtype=np.float32)
    rel_bias = np.asarray(rel_bias, dtype=np.float32)
    btabs = [_build_btab_near(rel_bias[0:HL]), _build_btab_near(rel_bias[HL:])]
    in_maps = []
    for c in range(8):
        b, half = c // 2, c % 2
        sl = slice(half * HD, (half + 1) * HD)
        in_maps.append({
            "xqT": np.ascontiguousarray(inputs_q[b].T).astype(
                ml_dtypes.bfloat16),
            "xkvT": np.ascontiguousarray(inputs_kv[b].T).astype(
                ml_dtypes.bfloat16),
            "wq": np.ascontiguousarray(Wq[:, sl]).astype(ml_dtypes.bfloat16),
            "wk": np.ascontiguousarray(Wk[:, sl]).astype(ml_dtypes.bfloat16),
            "wv": np.ascontiguousarray(Wv[:, sl]).astype(ml_dtypes.bfloat16),
            "wo": np.ascontiguousarray(Wo[sl, :]),
            "btab": btabs[half],
        })
    return in_maps


# -------------------------------------------------------------- device side

def _build_nc():
    nc = bacc.Bacc(None, target_bir_lowering=False)
    xqT_d = nc.dram_tensor("xqT", [E, S], BF16, kind="ExternalInput")
    xkvT_d = nc.dram_tensor("xkvT", [E, S], BF16, kind="ExternalInput")
    wq_d = nc.dram_tensor("wq", [E, HD], BF16, kind="ExternalInput")
    wk_d = nc.dram_tensor("wk", [E, HD], BF16, kind="ExternalInput")
    wv_d = nc.dram_tensor("wv", [E, HD], BF16, kind="ExternalInput")
    wo_d = nc.dram_tensor("wo", [HD, E], F32, kind="ExternalInput")
    btab_d = nc.dram_tensor("btab", [128, 4, 5, 2, 512], F8,
                            kind="ExternalInput")
    out_d = nc.dram_tensor("out", [S, E], F32, kind="ExternalOutput")
    rec_d = nc.dram_tensor("rec_scratch", [4, 4, 2, 512], F32)

    with tile.TileContext(nc) as tc:
        with (
            tc.tile_pool(name="const", bufs=1) as pconst,
            tc.tile_pool(name="persist", bufs=1) as pper,
        ):
            ident = pconst.tile([128, 128], F32)
            make_identity(nc, ident)
            identf8 = pconst.tile([128, 128], F8)
            nc.vector.tensor_copy(identf8, ident)

            qT = pper.tile([128, 4, S], BF16)         # [pair-dims, hp, tok]
            kT = pper.tile([128, 4, S], BF16)
            vA = pper.tile([128, NT, HL * 65], BF16)  # v + ones col per head

            vAr = vA.rearrange("p t (h c) -> p t h c", c=65)
            nc.vector.memset(vAr[:, :, :, 64:65], 1.0)

            b_sb = pper.tile([128, 4, 5, 2, 512], F8)
            wo_sb = pper.tile([128, 4, E], F32R)

            # ---------------- stage 1: projections from host-transposed x
            with (
                tc.tile_pool(name="s1w", bufs=1) as p1w,
                tc.tile_pool(name="s1xq", bufs=4) as p1xq,
                tc.tile_pool(name="s1xv", bufs=3) as p1xv,
                tc.tile_pool(name="psP", bufs=4, space="PSUM") as psP,
            ):
                wq_sb = p1w.tile([128, NE, HD], BF16)
                wk_sb = p1w.tile([128, NE, HD], BF16)
                wv_sb = p1w.tile([128, NE, HD], BF16)
                xqT_r = xqT_d[:].rearrange("(e p) s -> p e s", p=128)
                xkvT_r = xkvT_d[:].rearrange("(e p) s -> p e s", p=128)
                nc.sync.dma_start(
                    out=wq_sb, in_=wq_d[:].rearrange("(e p) n -> p e n", p=128))
                xTqs = []
                for tq in range(NQ):
                    xTq = p1xq.tile([128, NE, 512], BF16, tag="xq")
                    nc.sync.dma_start(
                        out=xTq, in_=xqT_r[:, :, tq * 512:(tq + 1) * 512])
                    xTqs.append(xTq)
                nc.sync.dma_start(
                    out=wk_sb, in_=wk_d[:].rearrange("(e p) n -> p e n", p=128))
                nc.sync.dma_start(
                    out=wv_sb, in_=wv_d[:].rearrange("(e p) n -> p e n", p=128))
                nc.sync.dma_start(out=b_sb, in_=btab_d[:])
                nc.sync.dma_start(
                    out=wo_sb,
                    in_=wo_d[:].bitcast(F32R).rearrange(
                        "(g p) n -> p g n", p=128))

                # pass A: q projection (bf16)
                for tq in range(NQ):
                    sl = slice(tq * 512, (tq + 1) * 512)
                    xTq = xTqs[tq]
                    for hc in range(4):
                        qps = psP.tile([128, 512], F32, tag="pj")
                        for e in range(NE):
                            nc.tensor.matmul(
                                qps,
                                wq_sb[:, e, hc * 128:(hc + 1) * 128],
                                xTq[:, e, :],
                                start=(e == 0), stop=(e == NE - 1))
                        nc.scalar.copy(qT[:, hc, sl], qps)

                # pass B: k and v projections (f32r, shared xT tile)
                for tq in range(NQ):
                    sl = slice(tq * 512, (tq + 1) * 512)
                    xTv = p1xv.tile([128, NE, 512], BF16, tag="xv")
                    nc.sync.dma_start(out=xTv, in_=xkvT_r[:, :, sl])
                    for hc in range(4):
                        kps = psP.tile([128, 512], F32, tag="pj")
                        for e in range(NE):
                            nc.tensor.matmul(
                                kps,
                                wk_sb[:, e, hc * 128:(hc + 1) * 128],
                                xTv[:, e, :],
                                start=(e == 0), stop=(e == NE - 1))
                        nc.scalar.copy(kT[:, hc, sl], kps)
                    for j in range(4):
                        t = tq * 4 + j
                        vps = psP.tile([128, HD], F32, tag="pj")
                        for e in range(NE):
                            nc.tensor.matmul(
                                vps, xTv[:, e, j * 128:(j + 1) * 128],
                                wv_sb[:, e, :],
                                start=(e == 0), stop=(e == NE - 1))
                        nc.vector.tensor_copy(
                            vAr[:, t, :, 0:64],
                            vps.rearrange("p (h c) -> p h c", c=64))

            # ---------------- stages 2+3
            with tc.tile_pool(name="s2per", bufs=1) as p2per:
                oT = p2per.tile([128, 4, S], F32R)

                with (
                    tc.tile_pool(name="s2p", bufs=4) as p2p,
                    tc.tile_pool(name="s2rec", bufs=4) as p2rc,
                    tc.tile_pool(name="s2rep", bufs=2) as p2rp,
                    tc.tile_pool(name="psS", bufs=3, space="PSUM") as psS,
                    tc.tile_pool(name="psO", bufs=2, space="PSUM") as psO,
                ):
                 for hp in range(4):
                    for qc in range(4):
                        o0 = psO.tile([65, 512], F32, tag="o")
                        o1 = psO.tile([65, 512], F32, tag="o")
                        nkb = 4 * qc + 4
                        h0, h1 = 2 * hp, 2 * hp + 1

                        def issue_pv(kb, p4, w0, o0=o0, o1=o1, nkb=nkb,
                                     h0=h0, h1=h1):
                            nc.tensor.matmul(
                                o0[:, w0:512],
                                vA[:, kb, h0 * 65:(h0 + 1) * 65],
                                p4[:, w0:512],
                                start=(kb == 0), stop=(kb == nkb - 1),
                                skip_group_check=(w0 > 0))
                            nc.tensor.matmul(
                                o1[:, w0:512],
                                vA[:, kb, h1 * 65:(h1 + 1) * 65],
                                p4[:, 512 + w0:1024],
                                start=(kb == 0), stop=(kb == nkb - 1),
                                skip_group_check=(w0 > 0))

                        pend = None
                        for kb in range(nkb):
                            mi = 4 * qc - kb + 3
                            s2 = psS.tile([128, 1024], F32, tag="s")
                            near = mi <= 4
                            # diagonal-straddling blocks (mi<=3) only touch
                            # queries q >= w0; skip the fully-masked columns
                            w0 = 128 * (3 - mi) if mi <= 3 else 0
                            if near:
                                for hh in range(2):
                                    nc.tensor.matmul(
                                        s2[:, hh * 512 + w0:hh * 512 + 512],
                                        identf8,
                                        b_sb[:, hp, mi, hh, w0:512],
                                        start=True, stop=False)
                            for hh in range(2):
                                nc.tensor.matmul(
                                    s2[:, hh * 512 + w0:hh * 512 + 512],
                                    kT[hh * 64:hh * 64 + 64, hp,
                                       kb * 128:(kb + 1) * 128],
                                    qT[hh * 64:hh * 64 + 64, hp,
                                       qc * 512 + w0:(qc + 1) * 512],
                                    start=not near, stop=True)
                            p4 = p2p.tile([128, 1024], BF16, tag="p")
                            s2v = s2.rearrange("p (h n) -> p h n", n=512)
                            p4v = p4.rearrange("p (h n) -> p h n", n=512)
                            nc.scalar.activation(p4v[:, :, w0:512],
                                                 s2v[:, :, w0:512],
                                                 AF.Exp, scale=0.125)
                            if pend is not None:
                                issue_pv(*pend)
                            pend = (kb, p4, w0)
                        issue_pv(*pend)
                        # epilogue: normalize + drain O^T
                        osts = []
                        for ops_o in (o0, o1):
                            ost = p2rc.tile([65, 512], F32, tag="ost")
                            nc.vector.tensor_copy(ost, ops_o)
                            osts.append(ost)
                        for hh in range(2):
                            ost = osts[hh]
                            nc.vector.reciprocal(ost[64:65, :], ost[64:65, :])
                            nc.sync.dma_start(out=rec_d[hp, qc, hh],
                                              in_=ost[64:65, :])
                            rep = p2rp.tile([64, 512], F32, tag="rep")
                            src = rec_d[hp, qc, hh, :]
                            nc.sync.dma_start(
                                out=rep,
                                in_=bass.AP(
                                    tensor=src.tensor, offset=src.offset,
                                    ap=[[0, 64]] + src.ap,
                                ))
                            nc.vector.tensor_tensor(
                                out=oT[hh * 64:(hh + 1) * 64, hp,
                                       qc * 512:(qc + 1) * 512],
                                in0=ost[0:64, :], in1=rep,
                                op=mybir.AluOpType.mult)

                # ---------------- stage 3: output projection
                with (
                    tc.tile_pool(name="s3o", bufs=3) as p3o,
                    tc.tile_pool(name="psF", bufs=4, space="PSUM") as psF,
                ):
                    for t in range(NT):
                        oev = p3o.tile([128, E], F32, tag="oev")
                        for ec in range(2):
                            ops = psF.tile([128, 512], F32, tag="ops")
                            for hp in range(4):
                                nc.tensor.matmul(
                                    ops, oT[:, hp, t * 128:(t + 1) * 128],
                                    wo_sb[:, hp, ec * 512:(ec + 1) * 512],
                                    start=(hp == 0), stop=(hp == 3))
                            nc.scalar.copy(
                                oev[:, ec * 512:(ec + 1) * 512], ops)
                        nc.sync.dma_start(
                            out=out_d[t * 128:(t + 1) * 128, :], in_=oev)

    nc.compile()
    return nc


def _get_nc():
    if "nc" not in _NC_CACHE:
        _NC_CACHE["nc"] = _build_nc()
    return _NC_CACHE["nc"]


def kernel(inputs_q, inputs_kv, mask, Wq, Wk, Wv, Wo, rel_bias):
    nc = _get_nc()
    in_maps = make_in_maps(inputs_q, inputs_kv, Wq, Wk, Wv, Wo, rel_bias)
    res = run_bass_kernel_spmd(nc, in_maps, core_ids=list(range(8)))
    out = np.stack(
        [res.results[2 * b]["out"] + res.results[2 * b + 1]["out"]
         for b in range(B)])
    return out.astype(np.float32)


# revision 19
# speedup vs baseline: 1.0045x; 1.0045x over previous
"""T5-style causal multi-head attention (B=4, S=2048, E=1024, H=16, D=64)
on 8 NeuronCores. Sharding: core c handles batch c//2 and head half c%2
(8 heads). Host sums the two row-parallel partial output projections per
batch.  HW exec ~394us (baseline 614us).

Key optimizations over the original:
- The T5 bias saturates at bucket 31 for distance >= 113; that far-field
  value is constant per head across all keys of a query row, so it
  CANCELS IN SOFTMAX. Far blocks need no bias at all; only near blocks
  (mi <= 4) add a shifted table 8*(bias[bucket]-bias[31]) (+ mask -240)
  via a bf16 identity-matmul PSUM preload. The near table for all 4 head
  pairs stays SBUF-resident (no per-hp DMA bubbles).
- x is transposed on the HOST; stage 1 DMAs x^T tiles directly and runs
  only projection matmuls (no PE transposes, no PSUM->SBUF copy storm).
  Q path in bf16, K/V in f32r.
- Both heads' scores accumulate in one 2-bank PSUM tile [128,1024]; ONE
  ACT instruction does exp for both heads. Diagonal-straddling blocks
  are trimmed to the valid query range (w0) in QK, exp and PV.
- PV is issued one block behind QK so the PE never stalls on the exp.
- Softmax denominators come free via a ones-column in the PV lhsT
  (m=65); per (hp,qc) the den row is reciprocated on DVE, broadcast via
  a DRAM round-trip, and fused into the O^T drain multiply.
"""
import sys

sys.path.insert(0, "/opt/trn_rl_repo")

import numpy as np
import ml_dtypes

import concourse.bass as bass
import concourse.mybir as mybir
import concourse.tile as tile
from concourse import bacc
from concourse.bass_utils import run_bass_kernel_spmd
from concourse.masks import make_identity

F32, F32R, BF16 = mybir.dt.float32, mybir.dt.float32r, mybir.dt.bfloat16
F8 = mybir.dt.float8e4
AF = mybir.ActivationFunctionType

B, S, E, H, D = 4, 2048, 1024, 16, 64
HL = H // 2          # heads per core
HD = HL * D          # 512, per-core head dims
NUM_BUCKETS, MAX_DISTANCE = 32, 128
NEG8 = np.float32(-240.0)   # min-ish of fp8 e4m3 (IEEE): kills exp after /8
NT = S // 128        # 16 token blocks
NE = E // 8 // 16    # placeholder; real NE below
NE = E // 128        # 8 embed chunks
NQ = 4               # token quads (512 tokens each)

_NC_CACHE = {}


# ---------------------------------------------------------------- host side

def _np_bucket(distance):
    """Mirror reference._relative_position_bucket for causal (distance>=0),
    float32 arithmetic like jnp."""
    max_exact = NUM_BUCKETS // 2  # 16
    is_small = distance < max_exact
    safe = np.maximum(distance, 1).astype(np.float32)
    log_scale = np.log(safe / np.float32(max_exact)).astype(np.float32) / np.float32(
        np.log(np.float32(MAX_DISTANCE / max_exact))
    )
    large = max_exact + (log_scale * np.float32(NUM_BUCKETS - max_exact)).astype(
        np.int32
    )
    large = np.minimum(large, NUM_BUCKETS - 1)
    return np.where(is_small, distance, large)


def _build_btab_near(rel_bias_half):
    """rel_bias_half [8, 32] -> near-table [128 k, 4 hp, 5 mi, 2 h, 512 q]
    fp8, holding 8*(bias[bucket] - bias[31]) for valid, -240 for masked.
    The -bias[31] shift is the constant far-field bias, which cancels in
    softmax. m-index mi = (4*qc - kb) + 3; only mi <= 4 blocks need it."""
    rb = np.asarray(rel_bias_half, dtype=np.float32)        # [8, 32]
    qq = np.arange(512)[None, :]
    kk = np.arange(128)[:, None]
    tiles = []
    for mi in range(5):
        m = mi - 3
        dd = 128 * m + qq - kk                              # [128, 512]
        bucket = _np_bucket(np.maximum(dd, 0))
        vals = 8.0 * (rb[:, bucket] - rb[:, 31][:, None, None])   # [8,128,512]
        vals = np.where(dd[None] >= 0, vals, NEG8)
        tiles.append(vals.astype(np.float32))
    t = np.stack(tiles, axis=0)                             # [5, 8h, 128, 512]
    t = t.reshape(5, 4, 2, 128, 512).transpose(3, 1, 0, 2, 4)  # [128,4,5,2,512]
    return np.ascontiguousarray(t).astype(ml_dtypes.float8_e4m3)


def make_in_maps(inputs_q, inputs_kv, Wq, Wk, Wv, Wo, rel_bias):
    inputs_q = np.asarray(inputs_q, dtype=np.float32)
    inputs_kv = np.asarray(inputs_kv, dtype=np.float32)
    Wq = np.asarray(Wq, dtype=np.float32)
    Wk = np.asarray(Wk, dtype=np.float32)
    Wv = np.asarray(Wv, dtype=np.float32)
    Wo = np.asarray(Wo, dtype=np.float32)
    rel_bias = np.asarray(rel_bias, dtype=np.float32)
    btabs = [_build_btab_near(rel_bias[0:HL]), _build_btab_near(rel_bias[HL:])]
    in_maps = []
    for c in range(8):
        b, half = c // 2, c % 2
        sl = slice(half * HD, (half + 1) * HD)
        in_maps.append({
            "xqT": np.ascontiguousarray(inputs_q[b].T).astype(
                ml_dtypes.bfloat16),
            "xkvT": np.ascontiguousarray(inputs_kv[b].T).astype(
                ml_dtypes.bfloat16),
            "wq": np.ascontiguousarray(Wq[:, sl]).astype(ml_dtypes.bfloat16),
            "wk": np.ascontiguousarray(Wk[:, sl]).astype(ml_dtypes.bfloat16),
            "wv": np.ascontiguousarray(Wv[:, sl]).astype(ml_dtypes.bfloat16),
            "wo": np.ascontiguousarray(Wo[sl, :]),
            "btab": btabs[half],
        })
    return in_maps


# -------------------------------------------------------------- device side

def _build_nc():
    nc = bacc.Bacc(None, target_bir_lowering=False)
    xqT_d = nc.dram_tensor("xqT", [E, S], BF16, kind="ExternalInput")
    xkvT_d = nc.dram_tensor("xkvT", [E, S], BF16, kind="ExternalInput")
    wq_d = nc.dram_tensor("wq", [E, HD], BF16, kind="ExternalInput")
    wk_d = nc.dram_tensor("wk", [E, HD], BF16, kind="ExternalInput")
    wv_d = nc.dram_tensor("wv", [E, HD], BF16, kind="ExternalInput")
    wo_d = nc.dram_tensor("wo", [HD, E], F32, kind="ExternalInput")
    btab_d = nc.dram_tensor("btab", [128, 4, 5, 2, 512], F8,
                            kind="ExternalInput")
    out_d = nc.dram_tensor("out", [S, E], F32, kind="ExternalOutput")
    rec_d = nc.dram_tensor("rec_scratch", [4, 4, 2, 512], F32)

    with tile.TileContext(nc) as tc:
        with (
            tc.tile_pool(name="const", bufs=1) as pconst,
            tc.tile_pool(name="persist", bufs=1) as pper,
        ):
            ident = pconst.tile([128, 128], F32)
            make_identity(nc, ident)
            identf8 = pconst.tile([128, 128], F8)
            nc.vector.tensor_copy(identf8, ident)

            qT = pper.tile([128, 4, S], BF16)         # [pair-dims, hp, tok]
            kT = pper.tile([128, 4, S], BF16)
            vA = pper.tile([128, NT, HL * 65], BF16)  # v + ones col per head

            vAr = vA.rearrange("p t (h c) -> p t h c", c=65)
            nc.vector.memset(vAr[:, :, :, 64:65], 1.0)

            b_sb = pper.tile([128, 4, 5, 2, 512], F8)
            wo_sb = pper.tile([128, 4, E], F32R)

            # ---------------- stage 1: projections from host-transposed x
            with (
                tc.tile_pool(name="s1w", bufs=1) as p1w,
                tc.tile_pool(name="s1xq", bufs=4) as p1xq,
                tc.tile_pool(name="s1xv", bufs=3) as p1xv,
                tc.tile_pool(name="psP", bufs=4, space="PSUM") as psP,
            ):
                wq_sb = p1w.tile([128, NE, HD], BF16)
                wk_sb = p1w.tile([128, NE, HD], BF16)
                wv_sb = p1w.tile([128, NE, HD], BF16)
                xqT_r = xqT_d[:].rearrange("(e p) s -> p e s", p=128)
                xkvT_r = xkvT_d[:].rearrange("(e p) s -> p e s", p=128)
                nc.sync.dma_start(
                    out=wq_sb, in_=wq_d[:].rearrange("(e p) n -> p e n", p=128))
                xTqs = []
                for tq in range(NQ):
                    xTq = p1xq.tile([128, NE, 512], BF16, tag="xq")
                    nc.sync.dma_start(
                        out=xTq, in_=xqT_r[:, :, tq * 512:(tq + 1) * 512])
                    xTqs.append(xTq)
                nc.sync.dma_start(
                    out=wk_sb, in_=wk_d[:].rearrange("(e p) n -> p e n", p=128))
                nc.sync.dma_start(
                    out=wv_sb, in_=wv_d[:].rearrange("(e p) n -> p e n", p=128))
                nc.sync.dma_start(out=b_sb, in_=btab_d[:])
                nc.sync.dma_start(
                    out=wo_sb,
                    in_=wo_d[:].bitcast(F32R).rearrange(
                        "(g p) n -> p g n", p=128))

                # pass A: q projection (bf16)
                for tq in range(NQ):
                    sl = slice(tq * 512, (tq + 1) * 512)
                    xTq = xTqs[tq]
                    for hc in range(4):
                        qps = psP.tile([128, 512], F32, tag="pj")
                        for e in range(NE):
                            nc.tensor.matmul(
                                qps,
                                wq_sb[:, e, hc * 128:(hc + 1) * 128],
                                xTq[:, e, :],
                                start=(e == 0), stop=(e == NE - 1))
                        nc.scalar.copy(qT[:, hc, sl], qps)

                # pass B: k and v projections (f32r, shared xT tile)
                for tq in range(NQ):
                    sl = slice(tq * 512, (tq + 1) * 512)
                    xTv = p1xv.tile([128, NE, 512], BF16, tag="xv")
                    nc.sync.dma_start(out=xTv, in_=xkvT_r[:, :, sl])
                    for hc in range(4):
                        kps = psP.tile([128, 512], F32, tag="pj")
                        for e in range(NE):
                            nc.tensor.matmul(
                                kps,
                                wk_sb[:, e, hc * 128:(hc + 1) * 128],
                                xTv[:, e, :],
                                start=(e == 0), stop=(e == NE - 1))
                        nc.scalar.copy(kT[:, hc, sl], kps)
                    for j in range(4):
                        t = tq * 4 + j
                        vps = psP.tile([128, HD], F32, tag="pj")
                        for e in range(NE):
                            nc.tensor.matmul(
                                vps, xTv[:, e, j * 128:(j + 1) * 128],
                                wv_sb[:, e, :],
                                start=(e == 0), stop=(e == NE - 1))
                        nc.vector.tensor_copy(
                            vAr[:, t, :, 0:64],
                            vps.rearrange("p (h c) -> p h c", c=64))

            # ---------------- stages 2+3
            with tc.tile_pool(name="s2per", bufs=1) as p2per:
                oT = p2per.tile([128, 4, S], F32R)

                with (
                    tc.tile_pool(name="s2p", bufs=4) as p2p,
                    tc.tile_pool(name="s2rec", bufs=4) as p2rc,
                    tc.tile_pool(name="s2rep", bufs=2) as p2rp,
                    tc.tile_pool(name="psS", bufs=2, space="PSUM") as psS,
                    tc.tile_pool(name="psO", bufs=4, space="PSUM") as psO,
                ):
                 for hp in range(4):
                    for qc in range(4):
                        o0 = psO.tile([65, 512], F32, tag="o")
                        o1 = psO.tile([65, 512], F32, tag="o")
                        nkb = 4 * qc + 4
                        h0, h1 = 2 * hp, 2 * hp + 1

                        def issue_pv(kb, p4, w0, o0=o0, o1=o1, nkb=nkb,
                                     h0=h0, h1=h1):
                            nc.tensor.matmul(
                                o0[:, w0:512],
                                vA[:, kb, h0 * 65:(h0 + 1) * 65],
                                p4[:, w0:512],
                                start=(kb == 0), stop=(kb == nkb - 1),
                                skip_group_check=(w0 > 0))
                            nc.tensor.matmul(
                                o1[:, w0:512],
                                vA[:, kb, h1 * 65:(h1 + 1) * 65],
                                p4[:, 512 + w0:1024],
                                start=(kb == 0), stop=(kb == nkb - 1),
                                skip_group_check=(w0 > 0))

                        pend = None
                        for kb in range(nkb):
                            mi = 4 * qc - kb + 3
                            s2 = psS.tile([128, 1024], F32, tag="s")
                            near = mi <= 4
                            # diagonal-straddling blocks (mi<=3) only touch
                            # queries q >= w0; skip the fully-masked columns
                            w0 = 128 * (3 - mi) if mi <= 3 else 0
                            if near:
                                for hh in range(2):
                                    nc.tensor.matmul(
                                        s2[:, hh * 512 + w0:hh * 512 + 512],
                                        identf8,
                                        b_sb[:, hp, mi, hh, w0:512],
                                        start=True, stop=False)
                            for hh in range(2):
                                nc.tensor.matmul(
                                    s2[:, hh * 512 + w0:hh * 512 + 512],
                                    kT[hh * 64:hh * 64 + 64, hp,
                                       kb * 128:(kb + 1) * 128],
                                    qT[hh * 64:hh * 64 + 64, hp,
                                       qc * 512 + w0:(qc + 1) * 512],
                                    start=not near, stop=True)
                            p4 = p2p.tile([128, 1024], BF16, tag="p")
                            s2v = s2.rearrange("p (h n) -> p h n", n=512)
                            p4v = p4.rearrange("p (h n) -> p h n", n=512)
                            nc.scalar.activation(p4v[:, :, w0:512],
                                                 s2v[:, :, w0:512],
                                                 AF.Exp, scale=0.125)
                            if pend is not None:
                                issue_pv(*pend)
                            pend = (kb, p4, w0)
                        issue_pv(*pend)
                        # epilogue: normalize + drain O^T
                        osts = []
                        for ops_o in (o0, o1):
                            ost = p2rc.tile([65, 512], F32, tag="ost")
                            nc.vector.tensor_copy(ost, ops_o)
                            osts.append(ost)
                        for hh in range(2):
                            ost = osts[hh]
                            nc.vector.reciprocal(ost[64:65, :], ost[64:65, :])
                            nc.sync.dma_start(out=rec_d[hp, qc, hh],
                                              in_=ost[64:65, :])
                            rep = p2rp.tile([64, 512], F32, tag="rep")
                            src = rec_d[hp, qc, hh, :]
                            nc.sync.dma_start(
                                out=rep,
                                in_=bass.AP(
                                    tensor=src.tensor, offset=src.offset,
                                    ap=[[0, 64]] + src.ap,
                                ))
                            nc.vector.tensor_tensor(
                                out=oT[hh * 64:(hh + 1) * 64, hp,
                                       qc * 512:(qc + 1) * 512],
                                in0=ost[0:64, :], in1=rep,
                                op=mybir.AluOpType.mult)

                # ---------------- stage 3: output projection
                with (
                    tc.tile_pool(name="s3o", bufs=3) as p3o,
                    tc.tile_pool(name="psF", bufs=4, space="PSUM") as psF,
                ):
                    for t in range(NT):
                        oev = p3o.tile([128, E], F32, tag="oev")
                        for ec in range(2):
                            ops = psF.tile([128, 512], F32, tag="ops")
                            for hp in range(4):
                                nc.tensor.matmul(
                                    ops, oT[:, hp, t * 128:(t + 1) * 128],
                                    wo_sb[:, hp, ec * 512:(ec + 1) * 512],
                                    start=(hp == 0), stop=(hp == 3))
                            nc.scalar.copy(
                                oev[:, ec * 512:(ec + 1) * 512], ops)
                        nc.sync.dma_start(
                            out=out_d[t * 128:(t + 1) * 128, :], in_=oev)

    nc.compile()
    return nc


def _get_nc():
    if "nc" not in _NC_CACHE:
        _NC_CACHE["nc"] = _build_nc()
    return _NC_CACHE["nc"]


def kernel(inputs_q, inputs_kv, mask, Wq, Wk, Wv, Wo, rel_bias):
    nc = _get_nc()
    in_maps = make_in_maps(inputs_q, inputs_kv, Wq, Wk, Wv, Wo, rel_bias)
    res = run_bass_kernel_spmd(nc, in_maps, core_ids=list(range(8)))
    out = np.stack(
        [res.results[2 * b]["out"] + res.results[2 * b + 1]["out"]
         for b in range(B)])
    return out.astype(np.float32)


# revision 21
# speedup vs baseline: 1.0398x; 1.0351x over previous
"""T5-style causal multi-head attention (B=4, S=2048, E=1024, H=16, D=64)
on 8 NeuronCores. Sharding: core c handles batch c//2 and head half c%2
(8 heads). Host sums the two row-parallel partial output projections per
batch.  HW exec ~394us (baseline 614us).

Key optimizations over the original:
- The T5 bias saturates at bucket 31 for distance >= 113; that far-field
  value is constant per head across all keys of a query row, so it
  CANCELS IN SOFTMAX. Far blocks need no bias at all; only near blocks
  (mi <= 4) add a shifted table 8*(bias[bucket]-bias[31]) (+ mask -240)
  via a bf16 identity-matmul PSUM preload. The near table for all 4 head
  pairs stays SBUF-resident (no per-hp DMA bubbles).
- x is transposed on the HOST; stage 1 DMAs x^T tiles directly and runs
  only projection matmuls (no PE transposes, no PSUM->SBUF copy storm).
  Q path in bf16, K/V in f32r.
- Both heads' scores accumulate in one 2-bank PSUM tile [128,1024]; ONE
  ACT instruction does exp for both heads. Diagonal-straddling blocks
  are trimmed to the valid query range (w0) in QK, exp and PV.
- PV is issued one block behind QK so the PE never stalls on the exp.
- Softmax denominators come free via a ones-column in the PV lhsT
  (m=65); per (hp,qc) the den row is reciprocated on DVE, broadcast via
  a DRAM round-trip, and fused into the O^T drain multiply.
"""
import sys

sys.path.insert(0, "/opt/trn_rl_repo")

import numpy as np
import ml_dtypes

import concourse.bass as bass
import concourse.mybir as mybir
import concourse.tile as tile
from concourse import bacc
from concourse.bass_utils import run_bass_kernel_spmd
from concourse.masks import make_identity

F32, F32R, BF16 = mybir.dt.float32, mybir.dt.float32r, mybir.dt.bfloat16
F8 = mybir.dt.float8e4
AF = mybir.ActivationFunctionType

B, S, E, H, D = 4, 2048, 1024, 16, 64
HL = H // 2          # heads per core
HD = HL * D          # 512, per-core head dims
NUM_BUCKETS, MAX_DISTANCE = 32, 128
NEG8 = np.float32(-240.0)   # min-ish of fp8 e4m3 (IEEE): kills exp after /8
NT = S // 128        # 16 token blocks
NE = E // 8 // 16    # placeholder; real NE below
NE = E // 128        # 8 embed chunks
NQ = 4               # token quads (512 tokens each)

_NC_CACHE = {}


# ---------------------------------------------------------------- host side

def _np_bucket(distance):
    """Mirror reference._relative_position_bucket for causal (distance>=0),
    float32 arithmetic like jnp."""
    max_exact = NUM_BUCKETS // 2  # 16
    is_small = distance < max_exact
    safe = np.maximum(distance, 1).astype(np.float32)
    log_scale = np.log(safe / np.float32(max_exact)).astype(np.float32) / np.float32(
        np.log(np.float32(MAX_DISTANCE / max_exact))
    )
    large = max_exact + (log_scale * np.float32(NUM_BUCKETS - max_exact)).astype(
        np.int32
    )
    large = np.minimum(large, NUM_BUCKETS - 1)
    return np.where(is_small, distance, large)


def _build_btab_near(rel_bias_half):
    """rel_bias_half [8, 32] -> near-table [128 k, 4 hp, 5 mi, 2 h, 512 q]
    fp8, holding 8*(bias[bucket] - bias[31]) for valid, -240 for masked.
    The -bias[31] shift is the constant far-field bias, which cancels in
    softmax. m-index mi = (4*qc - kb) + 3; only mi <= 4 blocks need it."""
    rb = np.asarray(rel_bias_half, dtype=np.float32)        # [8, 32]
    qq = np.arange(512)[None, :]
    kk = np.arange(128)[:, None]
    tiles = []
    for mi in range(5):
        m = mi - 3
        dd = 128 * m + qq - kk                              # [128, 512]
        bucket = _np_bucket(np.maximum(dd, 0))
        vals = 8.0 * (rb[:, bucket] - rb[:, 31][:, None, None])   # [8,128,512]
        vals = np.where(dd[None] >= 0, vals, NEG8)
        tiles.append(vals.astype(np.float32))
    t = np.stack(tiles, axis=0)                             # [5, 8h, 128, 512]
    t = t.reshape(5, 4, 2, 128, 512).transpose(3, 1, 0, 2, 4)  # [128,4,5,2,512]
    return np.ascontiguousarray(t).astype(ml_dtypes.float8_e4m3)


def make_in_maps(inputs_q, inputs_kv, Wq, Wk, Wv, Wo, rel_bias):
    inputs_q = np.asarray(inputs_q, dtype=np.float32)
    inputs_kv = np.asarray(inputs_kv, dtype=np.float32)
    Wq = np.asarray(Wq, dtype=np.float32)
    Wk = np.asarray(Wk, dtype=np.float32)
    Wv = np.asarray(Wv, dtype=np.float32)
    Wo = np.asarray(Wo, dtype=np.float32)
    rel_bias = np.asarray(rel_bias, dtype=np.float32)
    btabs = [_build_btab_near(rel_bias[0:HL]), _build_btab_near(rel_bias[HL:])]
    in_maps = []
    for c in range(8):
        b, half = c // 2, c % 2
        sl = slice(half * HD, (half + 1) * HD)
        in_maps.append({
            "xqT": np.ascontiguousarray(inputs_q[b].T).astype(
                ml_dtypes.bfloat16),
            "xkvT": np.ascontiguousarray(inputs_kv[b].T).astype(
                ml_dtypes.bfloat16),
            "wq": np.ascontiguousarray(Wq[:, sl]).astype(ml_dtypes.bfloat16),
            "wk": np.ascontiguousarray(Wk[:, sl]).astype(ml_dtypes.bfloat16),
            "wv": np.ascontiguousarray(Wv[:, sl]).astype(ml_dtypes.bfloat16),
            "wo": np.ascontiguousarray(Wo[sl, :]),
            "btab": btabs[half],
        })
    return in_maps


# -------------------------------------------------------------- device side

def _build_nc():
    nc = bacc.Bacc(None, target_bir_lowering=False)
    xqT_d = nc.dram_tensor("xqT", [E, S], BF16, kind="ExternalInput")
    xkvT_d = nc.dram_tensor("xkvT", [E, S], BF16, kind="ExternalInput")
    wq_d = nc.dram_tensor("wq", [E, HD], BF16, kind="ExternalInput")
    wk_d = nc.dram_tensor("wk", [E, HD], BF16, kind="ExternalInput")
    wv_d = nc.dram_tensor("wv", [E, HD], BF16, kind="ExternalInput")
    wo_d = nc.dram_tensor("wo", [HD, E], F32, kind="ExternalInput")
    btab_d = nc.dram_tensor("btab", [128, 4, 5, 2, 512], F8,
                            kind="ExternalInput")
    out_d = nc.dram_tensor("out", [S, E], F32, kind="ExternalOutput")
    rec_d = nc.dram_tensor("rec_scratch", [4, 4, 2, 512], F32)

    with tile.TileContext(nc) as tc:
        with (
            tc.tile_pool(name="const", bufs=1) as pconst,
            tc.tile_pool(name="persist", bufs=1) as pper,
        ):
            ident = pconst.tile([128, 128], F32)
            make_identity(nc, ident)
            identf8 = pconst.tile([128, 128], F8)
            nc.vector.tensor_copy(identf8, ident)

            qT = pper.tile([128, 4, S], BF16)         # [pair-dims, hp, tok]
            kT = pper.tile([128, 4, S], BF16)
            vA = pper.tile([128, NT, HL * 65], BF16)  # v + ones col per head

            vAr = vA.rearrange("p t (h c) -> p t h c", c=65)
            nc.vector.memset(vAr[:, :, :, 64:65], 1.0)

            b_sb = pper.tile([128, 4, 5, 2, 512], F8)
            wo_sb = pper.tile([128, 4, E], F32R)

            # ---------------- stage 1: projections from host-transposed x
            with (
                tc.tile_pool(name="s1w", bufs=1) as p1w,
                tc.tile_pool(name="s1xq", bufs=4) as p1xq,
                tc.tile_pool(name="s1xv", bufs=4) as p1xv,
                tc.tile_pool(name="psP", bufs=4, space="PSUM") as psP,
            ):
                wq_sb = p1w.tile([128, NE, HD], BF16)
                wk_sb = p1w.tile([128, NE, HD], BF16)
                wv_sb = p1w.tile([128, NE, HD], BF16)
                xqT_r = xqT_d[:].rearrange("(e p) s -> p e s", p=128)
                xkvT_r = xkvT_d[:].rearrange("(e p) s -> p e s", p=128)
                nc.sync.dma_start(
                    out=wq_sb, in_=wq_d[:].rearrange("(e p) n -> p e n", p=128))
                xTqs = []
                for tq in range(NQ):
                    xTq = p1xq.tile([128, NE, 512], BF16, tag="xq")
                    if tq == 0:
                        # per-chunk DMAs: the first matmul only needs e=0,
                        # so don't make it wait for the whole 1MB tile
                        for e in range(NE):
                            nc.sync.dma_start(
                                out=xTq[:, e, :], in_=xqT_r[:, e, 0:512])
                    else:
                        nc.sync.dma_start(
                            out=xTq, in_=xqT_r[:, :, tq * 512:(tq + 1) * 512])
                    xTqs.append(xTq)
                nc.sync.dma_start(
                    out=wk_sb, in_=wk_d[:].rearrange("(e p) n -> p e n", p=128))
                nc.sync.dma_start(
                    out=wv_sb, in_=wv_d[:].rearrange("(e p) n -> p e n", p=128))
                nc.sync.dma_start(out=b_sb, in_=btab_d[:])
                nc.sync.dma_start(
                    out=wo_sb,
                    in_=wo_d[:].bitcast(F32R).rearrange(
                        "(g p) n -> p g n", p=128))

                # pass A: q projection (bf16)
                for tq in range(NQ):
                    sl = slice(tq * 512, (tq + 1) * 512)
                    xTq = xTqs[tq]
                    for hc in range(4):
                        qps = psP.tile([128, 512], F32, tag="pj")
                        for e in range(NE):
                            nc.tensor.matmul(
                                qps,
                                wq_sb[:, e, hc * 128:(hc + 1) * 128],
                                xTq[:, e, :],
                                start=(e == 0), stop=(e == NE - 1))
                        nc.scalar.copy(qT[:, hc, sl], qps)

                # pass B: k and v projections (f32r, shared xT tile)
                for tq in range(NQ):
                    sl = slice(tq * 512, (tq + 1) * 512)
                    xTv = p1xv.tile([128, NE, 512], BF16, tag="xv")
                    nc.sync.dma_start(out=xTv, in_=xkvT_r[:, :, sl])
                    for hc in range(4):
                        kps = psP.tile([128, 512], F32, tag="pj")
                        for e in range(NE):
                            nc.tensor.matmul(
                                kps,
                                wk_sb[:, e, hc * 128:(hc + 1) * 128],
                                xTv[:, e, :],
                                start=(e == 0), stop=(e == NE - 1))
                        nc.scalar.copy(kT[:, hc, sl], kps)
                    for j in range(4):
                        t = tq * 4 + j
                        vps = psP.tile([128, HD], F32, tag="pj")
                        for e in range(NE):
                            nc.tensor.matmul(
                                vps, xTv[:, e, j * 128:(j + 1) * 128],
                                wv_sb[:, e, :],
                                start=(e == 0), stop=(e == NE - 1))
                        nc.vector.tensor_copy(
                            vAr[:, t, :, 0:64],
                            vps.rearrange("p (h c) -> p h c", c=64))

            # ---------------- stages 2+3
            with tc.tile_pool(name="s2per", bufs=1) as p2per:
                oT = p2per.tile([128, 4, S], F32R)

                with (
                    tc.tile_pool(name="s2p", bufs=3) as p2p,
                    tc.tile_pool(name="s2rec", bufs=3) as p2rc,
                    tc.tile_pool(name="s2rep", bufs=2) as p2rp,
                    tc.tile_pool(name="psS", bufs=2, space="PSUM") as psS,
                    tc.tile_pool(name="psO", bufs=4, space="PSUM") as psO,
                ):
                 for hp in range(4):
                    for qc in range(4):
                        o0 = psO.tile([65, 512], F32, tag="o")
                        o1 = psO.tile([65, 512], F32, tag="o")
                        nkb = 4 * qc + 4
                        h0, h1 = 2 * hp, 2 * hp + 1

                        def issue_pv(kb, p4, w0, o0=o0, o1=o1, nkb=nkb,
                                     h0=h0, h1=h1):
                            nc.tensor.matmul(
                                o0[:, w0:512],
                                vA[:, kb, h0 * 65:(h0 + 1) * 65],
                                p4[:, w0:512],
                                start=(kb == 0), stop=(kb == nkb - 1),
                                skip_group_check=(w0 > 0))
                            nc.tensor.matmul(
                                o1[:, w0:512],
                                vA[:, kb, h1 * 65:(h1 + 1) * 65],
                                p4[:, 512 + w0:1024],
                                start=(kb == 0), stop=(kb == nkb - 1),
                                skip_group_check=(w0 > 0))

                        pend = None
                        for kb in range(nkb):
                            mi = 4 * qc - kb + 3
                            s2 = psS.tile([128, 1024], F32, tag="s")
                            near = mi <= 4
                            # diagonal-straddling blocks (mi<=3) only touch
                            # queries q >= w0; skip the fully-masked columns
                            w0 = 128 * (3 - mi) if mi <= 3 else 0
                            if near:
                                for hh in range(2):
                                    nc.tensor.matmul(
                                        s2[:, hh * 512 + w0:hh * 512 + 512],
                                        identf8,
                                        b_sb[:, hp, mi, hh, w0:512],
                                        start=True, stop=False)
                            for hh in range(2):
                                nc.tensor.matmul(
                                    s2[:, hh * 512 + w0:hh * 512 + 512],
                                    kT[hh * 64:hh * 64 + 64, hp,
                                       kb * 128:(kb + 1) * 128],
                                    qT[hh * 64:hh * 64 + 64, hp,
                                       qc * 512 + w0:(qc + 1) * 512],
                                    start=not near, stop=True)
                            p4 = p2p.tile([128, 1024], BF16, tag="p")
                            s2v = s2.rearrange("p (h n) -> p h n", n=512)
                            p4v = p4.rearrange("p (h n) -> p h n", n=512)
                            nc.scalar.activation(p4v[:, :, w0:512],
                                                 s2v[:, :, w0:512],
                                                 AF.Exp, scale=0.125)
                            if pend is not None:
                                issue_pv(*pend)
                            pend = (kb, p4, w0)
                        issue_pv(*pend)
                        # epilogue: normalize + drain O^T
                        for hh, ops_o in ((0, o0), (1, o1)):
                            ost = p2rc.tile([65, 512], F32, tag="ost")
                            nc.vector.tensor_copy(ost, ops_o)
                            nc.vector.reciprocal(ost[64:65, :], ost[64:65, :])
                            nc.sync.dma_start(out=rec_d[hp, qc, hh],
                                              in_=ost[64:65, :])
                            rep = p2rp.tile([64, 512], F32, tag="rep")
                            src = rec_d[hp, qc, hh, :]
                            nc.sync.dma_start(
                                out=rep,
                                in_=bass.AP(
                                    tensor=src.tensor, offset=src.offset,
                                    ap=[[0, 64]] + src.ap,
                                ))
                            nc.vector.tensor_tensor(
                                out=oT[hh * 64:(hh + 1) * 64, hp,
                                       qc * 512:(qc + 1) * 512],
                                in0=ost[0:64, :], in1=rep,
                                op=mybir.AluOpType.mult)

                # ---------------- stage 3: output projection
                with (
                    tc.tile_pool(name="s3o", bufs=3) as p3o,
                    tc.tile_pool(name="psF", bufs=4, space="PSUM") as psF,
                ):
                    for t in range(NT):
                        oev = p3o.tile([128, E], F32, tag="oev")
                        for ec in range(2):
                            ops = psF.tile([128, 512], F32, tag="ops")
                            for hp in range(4):
                                nc.tensor.matmul(
                                    ops, oT[:, hp, t * 128:(t + 1) * 128],
                                    wo_sb[:, hp, ec * 512:(ec + 1) * 512],
                                    start=(hp == 0), stop=(hp == 3))
                            nc.scalar.copy(
                                oev[:, ec * 512:(ec + 1) * 512], ops)
                        nc.sync.dma_start(
                            out=out_d[t * 128:(t + 1) * 128, :], in_=oev)

    nc.compile()
    return nc


def _get_nc():
    if "nc" not in _NC_CACHE:
        _NC_CACHE["nc"] = _build_nc()
    return _NC_CACHE["nc"]


def kernel(inputs_q, inputs_kv, mask, Wq, Wk, Wv, Wo, rel_bias):
    nc = _get_nc()
    in_maps = make_in_maps(inputs_q, inputs_kv, Wq, Wk, Wv, Wo, rel_bias)
    res = run_bass_kernel_spmd(nc, in_maps, core_ids=list(range(8)))
    out = np.stack(
        [res.results[2 * b]["out"] + res.results[2 * b + 1]["out"]
         for b in range(B)])
    return out.astype(np.float32)


# revision 23
# speedup vs baseline: 1.0520x; 1.0118x over previous
"""T5-style causal multi-head attention (B=4, S=2048, E=1024, H=16, D=64)
on 8 NeuronCores. Sharding: core c handles batch c//2 and head half c%2
(8 heads). Host sums the two row-parallel partial output projections per
batch.  HW exec ~394us (baseline 614us).

Key optimizations over the original:
- The T5 bias saturates at bucket 31 for distance >= 113; that far-field
  value is constant per head across all keys of a query row, so it
  CANCELS IN SOFTMAX. Far blocks need no bias at all; only near blocks
  (mi <= 4) add a shifted table 8*(bias[bucket]-bias[31]) (+ mask -240)
  via a bf16 identity-matmul PSUM preload. The near table for all 4 head
  pairs stays SBUF-resident (no per-hp DMA bubbles).
- x is transposed on the HOST; stage 1 DMAs x^T tiles directly and runs
  only projection matmuls (no PE transposes, no PSUM->SBUF copy storm).
  Q path in bf16, K/V in f32r.
- Both heads' scores accumulate in one 2-bank PSUM tile [128,1024]; ONE
  ACT instruction does exp for both heads. Diagonal-straddling blocks
  are trimmed to the valid query range (w0) in QK, exp and PV.
- PV is issued one block behind QK so the PE never stalls on the exp.
- Softmax denominators come free via a ones-column in the PV lhsT
  (m=65); per (hp,qc) the den row is reciprocated on DVE, broadcast via
  a DRAM round-trip, and fused into the O^T drain multiply.
"""
import sys

sys.path.insert(0, "/opt/trn_rl_repo")

import numpy as np
import ml_dtypes

import concourse.bass as bass
import concourse.mybir as mybir
import concourse.tile as tile
from concourse import bacc
from concourse.bass_utils import run_bass_kernel_spmd
from concourse.masks import make_identity

F32, F32R, BF16 = mybir.dt.float32, mybir.dt.float32r, mybir.dt.bfloat16
F8 = mybir.dt.float8e4
AF = mybir.ActivationFunctionType

B, S, E, H, D = 4, 2048, 1024, 16, 64
HL = H // 2          # heads per core
HD = HL * D          # 512, per-core head dims
NUM_BUCKETS, MAX_DISTANCE = 32, 128
NEG8 = np.float32(-240.0)   # min-ish of fp8 e4m3 (IEEE): kills exp after /8
NT = S // 128        # 16 token blocks
NE = E // 8 // 16    # placeholder; real NE below
NE = E // 128        # 8 embed chunks
NQ = 4               # token quads (512 tokens each)

_NC_CACHE = {}


# ---------------------------------------------------------------- host side

def _np_bucket(distance):
    """Mirror reference._relative_position_bucket for causal (distance>=0),
    float32 arithmetic like jnp."""
    max_exact = NUM_BUCKETS // 2  # 16
    is_small = distance < max_exact
    safe = np.maximum(distance, 1).astype(np.float32)
    log_scale = np.log(safe / np.float32(max_exact)).astype(np.float32) / np.float32(
        np.log(np.float32(MAX_DISTANCE / max_exact))
    )
    large = max_exact + (log_scale * np.float32(NUM_BUCKETS - max_exact)).astype(
        np.int32
    )
    large = np.minimum(large, NUM_BUCKETS - 1)
    return np.where(is_small, distance, large)


def _build_btab_near(rel_bias_half):
    """rel_bias_half [8, 32] -> near-table [128 k, 4 hp, 5 mi, 2 h, 512 q]
    fp8, holding 8*(bias[bucket] - bias[31]) for valid, -240 for masked.
    The -bias[31] shift is the constant far-field bias, which cancels in
    softmax. m-index mi = (4*qc - kb) + 3; only mi <= 4 blocks need it."""
    rb = np.asarray(rel_bias_half, dtype=np.float32)        # [8, 32]
    qq = np.arange(512)[None, :]
    kk = np.arange(128)[:, None]
    tiles = []
    for mi in range(5):
        m = mi - 3
        dd = 128 * m + qq - kk                              # [128, 512]
        bucket = _np_bucket(np.maximum(dd, 0))
        vals = 8.0 * (rb[:, bucket] - rb[:, 31][:, None, None])   # [8,128,512]
        vals = np.where(dd[None] >= 0, vals, NEG8)
        tiles.append(vals.astype(np.float32))
    t = np.stack(tiles, axis=0)                             # [5, 8h, 128, 512]
    t = t.reshape(5, 4, 2, 128, 512).transpose(3, 1, 0, 2, 4)  # [128,4,5,2,512]
    return np.ascontiguousarray(t).astype(ml_dtypes.float8_e4m3)


def make_in_maps(inputs_q, inputs_kv, Wq, Wk, Wv, Wo, rel_bias):
    inputs_q = np.asarray(inputs_q, dtype=np.float32)
    inputs_kv = np.asarray(inputs_kv, dtype=np.float32)
    Wq = np.asarray(Wq, dtype=np.float32)
    Wk = np.asarray(Wk, dtype=np.float32)
    Wv = np.asarray(Wv, dtype=np.float32)
    Wo = np.asarray(Wo, dtype=np.float32)
    rel_bias = np.asarray(rel_bias, dtype=np.float32)
    btabs = [_build_btab_near(rel_bias[0:HL]), _build_btab_near(rel_bias[HL:])]
    in_maps = []
    for c in range(8):
        b, half = c // 2, c % 2
        sl = slice(half * HD, (half + 1) * HD)
        in_maps.append({
            "xqT": np.ascontiguousarray(inputs_q[b].T).astype(
                ml_dtypes.bfloat16),
            "xkvT": np.ascontiguousarray(inputs_kv[b].T).astype(
                ml_dtypes.bfloat16),
            "wq": np.ascontiguousarray(Wq[:, sl]).astype(ml_dtypes.bfloat16),
            "wk": np.ascontiguousarray(Wk[:, sl]).astype(ml_dtypes.bfloat16),
            "wv": np.ascontiguousarray(Wv[:, sl]).astype(ml_dtypes.bfloat16),
            "wo": np.ascontiguousarray(Wo[sl, :]),
            "btab": btabs[half],
        })
    return in_maps


# -------------------------------------------------------------- device side

def _build_nc():
    nc = bacc.Bacc(None, target_bir_lowering=False)
    xqT_d = nc.dram_tensor("xqT", [E, S], BF16, kind="ExternalInput")
    xkvT_d = nc.dram_tensor("xkvT", [E, S], BF16, kind="ExternalInput")
    wq_d = nc.dram_tensor("wq", [E, HD], BF16, kind="ExternalInput")
    wk_d = nc.dram_tensor("wk", [E, HD], BF16, kind="ExternalInput")
    wv_d = nc.dram_tensor("wv", [E, HD], BF16, kind="ExternalInput")
    wo_d = nc.dram_tensor("wo", [HD, E], F32, kind="ExternalInput")
    btab_d = nc.dram_tensor("btab", [128, 4, 5, 2, 512], F8,
                            kind="ExternalInput")
    out_d = nc.dram_tensor("out", [S, E], F32, kind="ExternalOutput")
    rec_d = nc.dram_tensor("rec_scratch", [4, 4, 2, 512], F32)

    with tile.TileContext(nc) as tc:
        with (
            tc.tile_pool(name="const", bufs=1) as pconst,
            tc.tile_pool(name="persist", bufs=1) as pper,
        ):
            ident = pconst.tile([128, 128], F32)
            make_identity(nc, ident)
            identf8 = pconst.tile([128, 128], F8)
            nc.vector.tensor_copy(identf8, ident)

            qT = pper.tile([128, 4, S], BF16)         # [pair-dims, hp, tok]
            kT = pper.tile([128, 4, S], BF16)
            vA = pper.tile([128, NT, HL * 65], BF16)  # v + ones col per head

            vAr = vA.rearrange("p t (h c) -> p t h c", c=65)
            nc.vector.memset(vAr[:, :, :, 64:65], 1.0)

            b_sb = pper.tile([128, 4, 5, 2, 512], F8)
            wo_sb = pper.tile([128, 4, E], F32R)

            # ---------------- stage 1: projections from host-transposed x
            with (
                tc.tile_pool(name="s1w", bufs=1) as p1w,
                tc.tile_pool(name="s1xq", bufs=4) as p1xq,
                tc.tile_pool(name="s1xv", bufs=4) as p1xv,
                tc.tile_pool(name="psP", bufs=4, space="PSUM") as psP,
            ):
                wq_sb = p1w.tile([128, NE, HD], BF16)
                wk_sb = p1w.tile([128, NE, HD], BF16)
                wv_sb = p1w.tile([128, NE, HD], BF16)
                xqT_r = xqT_d[:].rearrange("(e p) s -> p e s", p=128)
                xkvT_r = xkvT_d[:].rearrange("(e p) s -> p e s", p=128)
                nc.sync.dma_start(
                    out=wq_sb, in_=wq_d[:].rearrange("(e p) n -> p e n", p=128))
                xTqs = []
                for tq in range(NQ):
                    xTq = p1xq.tile([128, NE, 512], BF16, tag="xq")
                    if tq == 0:
                        # per-chunk DMAs: the first matmul only needs e=0,
                        # so don't make it wait for the whole 1MB tile
                        for e in range(NE):
                            nc.sync.dma_start(
                                out=xTq[:, e, :], in_=xqT_r[:, e, 0:512])
                    else:
                        nc.sync.dma_start(
                            out=xTq, in_=xqT_r[:, :, tq * 512:(tq + 1) * 512])
                    xTqs.append(xTq)
                nc.sync.dma_start(
                    out=wk_sb, in_=wk_d[:].rearrange("(e p) n -> p e n", p=128))
                nc.sync.dma_start(
                    out=wv_sb, in_=wv_d[:].rearrange("(e p) n -> p e n", p=128))
                nc.sync.dma_start(out=b_sb, in_=btab_d[:])
                nc.sync.dma_start(
                    out=wo_sb,
                    in_=wo_d[:].bitcast(F32R).rearrange(
                        "(g p) n -> p g n", p=128))

                # pass A: q projection (bf16)
                for tq in range(NQ):
                    sl = slice(tq * 512, (tq + 1) * 512)
                    xTq = xTqs[tq]
                    for hc in range(4):
                        qps = psP.tile([128, 512], F32, tag="pj")
                        for e in range(NE):
                            nc.tensor.matmul(
                                qps,
                                wq_sb[:, e, hc * 128:(hc + 1) * 128],
                                xTq[:, e, :],
                                start=(e == 0), stop=(e == NE - 1))
                        nc.scalar.copy(qT[:, hc, sl], qps)

                # pass B: k and v projections (f32r, shared xT tile)
                for tq in range(NQ):
                    sl = slice(tq * 512, (tq + 1) * 512)
                    xTv = p1xv.tile([128, NE, 512], BF16, tag="xv")
                    nc.sync.dma_start(out=xTv, in_=xkvT_r[:, :, sl])
                    for hc in range(4):
                        kps = psP.tile([128, 512], F32, tag="pj")
                        for e in range(NE):
                            nc.tensor.matmul(
                                kps,
                                wk_sb[:, e, hc * 128:(hc + 1) * 128],
                                xTv[:, e, :],
                                start=(e == 0), stop=(e == NE - 1))
                        nc.scalar.copy(kT[:, hc, sl], kps)
                    for j in range(4):
                        t = tq * 4 + j
                        vps = psP.tile([128, HD], F32, tag="pj")
                        for e in range(NE):
                            nc.tensor.matmul(
                                vps, xTv[:, e, j * 128:(j + 1) * 128],
                                wv_sb[:, e, :],
                                start=(e == 0), stop=(e == NE - 1))
                        nc.vector.tensor_copy(
                            vAr[:, t, :, 0:64],
                            vps.rearrange("p (h c) -> p h c", c=64))

            # ---------------- stages 2+3
            with tc.tile_pool(name="s2per", bufs=1) as p2per:
                oT = p2per.tile([128, 4, S], F32R)

                with (
                    tc.tile_pool(name="s2p", bufs=3) as p2p,
                    tc.tile_pool(name="s2rec", bufs=3) as p2rc,
                    tc.tile_pool(name="s2rep", bufs=2) as p2rp,
                    tc.tile_pool(name="psS", bufs=2, space="PSUM") as psS,
                    tc.tile_pool(name="psO", bufs=4, space="PSUM") as psO,
                ):
                 for hp in range(4):
                    for qc in range(4):
                        o0 = psO.tile([65, 512], F32, tag="o")
                        o1 = psO.tile([65, 512], F32, tag="o")
                        nkb = 4 * qc + 4
                        h0, h1 = 2 * hp, 2 * hp + 1

                        def issue_pv(kb, p4, w0, o0=o0, o1=o1, nkb=nkb,
                                     h0=h0, h1=h1):
                            nc.tensor.matmul(
                                o0[:, w0:512],
                                vA[:, kb, h0 * 65:(h0 + 1) * 65],
                                p4[:, w0:512],
                                start=(kb == 0), stop=(kb == nkb - 1),
                                skip_group_check=(w0 > 0))
                            nc.tensor.matmul(
                                o1[:, w0:512],
                                vA[:, kb, h1 * 65:(h1 + 1) * 65],
                                p4[:, 512 + w0:1024],
                                start=(kb == 0), stop=(kb == nkb - 1),
                                skip_group_check=(w0 > 0))

                        pend = None
                        for kb in range(nkb):
                            mi = 4 * qc - kb + 3
                            s2 = psS.tile([128, 1024], F32, tag="s")
                            near = mi <= 4
                            # diagonal-straddling blocks (mi<=3) only touch
                            # queries q >= w0; skip the fully-masked columns
                            w0 = 128 * (3 - mi) if mi <= 3 else 0
                            if near:
                                for hh in range(2):
                                    nc.tensor.matmul(
                                        s2[:, hh * 512 + w0:hh * 512 + 512],
                                        identf8,
                                        b_sb[:, hp, mi, hh, w0:512],
                                        start=True, stop=False)
                            for hh in range(2):
                                nc.tensor.matmul(
                                    s2[:, hh * 512 + w0:hh * 512 + 512],
                                    kT[hh * 64:hh * 64 + 64, hp,
                                       kb * 128:(kb + 1) * 128],
                                    qT[hh * 64:hh * 64 + 64, hp,
                                       qc * 512 + w0:(qc + 1) * 512],
                                    start=not near, stop=True)
                            p4 = p2p.tile([128, 1024], BF16, tag="p")
                            s2v = s2.rearrange("p (h n) -> p h n", n=512)
                            p4v = p4.rearrange("p (h n) -> p h n", n=512)
                            nc.scalar.activation(p4v[:, :, w0:512],
                                                 s2v[:, :, w0:512],
                                                 AF.Exp, scale=0.125)
                            if pend is not None:
                                issue_pv(*pend)
                            pend = (kb, p4, w0)
                        issue_pv(*pend)
                        # epilogue: normalize + drain O^T
                        for hh, ops_o in ((0, o0), (1, o1)):
                            ost = p2rc.tile([65, 512], F32, tag="ost")
                            nc.vector.tensor_copy(ost, ops_o)
                            nc.vector.reciprocal(ost[64:65, :], ost[64:65, :])
                            nc.sync.dma_start(out=rec_d[hp, qc, hh],
                                              in_=ost[64:65, :])
                            rep = p2rp.tile([64, 512], F32, tag="rep")
                            src = rec_d[hp, qc, hh, :]
                            nc.sync.dma_start(
                                out=rep,
                                in_=bass.AP(
                                    tensor=src.tensor, offset=src.offset,
                                    ap=[[0, 64]] + src.ap,
                                ))
                            nc.vector.tensor_tensor(
                                out=oT[hh * 64:(hh + 1) * 64, hp,
                                       qc * 512:(qc + 1) * 512],
                                in0=ost[0:64, :], in1=rep,
                                op=mybir.AluOpType.mult)

                # ---------------- stage 3: output projection
                with (
                    tc.tile_pool(name="s3o", bufs=3) as p3o,
                    tc.tile_pool(name="psF", bufs=4, space="PSUM") as psF,
                ):
                    for t in range(NT):
                        oev = p3o.tile([128, E], F32, tag="oev")
                        for ec in range(2):
                            ops = psF.tile([128, 512], F32, tag="ops")
                            for hp in range(4):
                                nc.tensor.matmul(
                                    ops, oT[:, hp, t * 128:(t + 1) * 128],
                                    wo_sb[:, hp, ec * 512:(ec + 1) * 512],
                                    start=(hp == 0), stop=(hp == 3))
                            nc.scalar.copy(
                                oev[:, ec * 512:(ec + 1) * 512], ops)
                        nc.sync.dma_start(
                            out=out_d[t * 128:(t + 1) * 128, :], in_=oev)

    nc.compile()
    return nc


def _get_nc():
    if "nc" not in _NC_CACHE:
        _NC_CACHE["nc"] = _build_nc()
    return _NC_CACHE["nc"]


def kernel(inputs_q, inputs_kv, mask, Wq, Wk, Wv, Wo, rel_bias):
    nc = _get_nc()
    in_maps = make_in_maps(inputs_q, inputs_kv, Wq, Wk, Wv, Wo, rel_bias)
    res = run_bass_kernel_spmd(nc, in_maps, core_ids=list(range(8)))
    out = np.stack(
        [res.results[2 * b]["out"] + res.results[2 * b + 1]["out"]
         for b in range(B)])
    return out.astype(np.float32)
